# revision 9
# baseline (speedup 1.0000x reference)
"""GINEConv x3 GNN backbone on 8 Trainium2 NeuronCores.

Strategy (graph-parallel by destination node, per the sharding hint):
- Nodes split contiguously across 8 cores (12500 each). Each core owns all
  edges whose destination is in its range, so segment-sum is core-local.
- Edge phase per chunk of 128 edges (layers 2/3):
    gather x[src]   : bulk dma_gather (q7 ucode; int16 idx) - node ids are
                      bucketed into 32768-row table slices to fit int16
    e = attr @ We+be: PE matmul, lhsT=attr3 [3,128], rhs=We3 [3,128]
    s = x_src + e   : DVE tensor_tensor add (PSUM + SBUF)
    msg = relu(s)   : ACT (bf16 out)
    onehot[dst]     : DVE tensor_tensor is_equal vs iota row (bf16)
    aggT += msg.T @ onehot : PE matmul accumulating in PSUM, feature-major,
                      flushed to an SBUF accumulator per (bucket, window)
- Node phase per 4-window group: zT = aggT + hT_own; hT = W.T @ zT (PE);
  leaky-relu = relu(z+b) - 0.01*relu(-z-b) (ACT+DVE); PE transpose to
  node-major; DMA to DRAM.
- AllGather of h after layers 1 and 2 so every core can gather any node row.
- Layer 1 (in_dim=5): host precomputes the 5-dim pre-activation messages
  x[src] + lin_edge(attr); device does relu + scatter + node phase.
- Edges are padded so every (bucket, window) run is a whole number of
  128-edge chunks, with the same chunk count on every core (single SPMD
  program). Pad edges gather row 0 and carry dst_local=-1 so their one-hot
  column is all zeros.
"""

import ml_dtypes
import numpy as np

import concourse.bacc as bacc
import concourse.bass as bass
import concourse.mybir as mybir
import concourse.tile as tile
from concourse.masks import make_identity

P = 128
BKT = 32768  # int16 bucket size for dma_gather indices
GB = 8  # chunks per dma_gather call (1024 idx = HW-validated max)
F32 = mybir.dt.float32
BF16 = mybir.dt.bfloat16
I16 = mybir.dt.int16
I32 = mybir.dt.int32

CFG = dict(n_nodes=100000, n_edges=1600000, in_dim=5, hid=128, n_cores=8)

GRP = 4  # windows per node-phase group
EPB = 4  # chunks per e-matmul/add/relu batch


def _ceil_div(a, b):
    return (a + b - 1) // b


def _host_prep(x, edge_index, edge_attr, params, cfg):
    """Returns (in_maps, cpw1, cpw_b)."""
    n_nodes = cfg["n_nodes"]
    n_cores = cfg["n_cores"]
    in_dim = cfg["in_dim"]
    hid = cfg["hid"]
    npc = n_nodes // n_cores
    nwin = _ceil_div(npc, P)
    nbkt = _ceil_div(n_nodes, BKT)

    x = np.asarray(x, np.float32)
    src = np.asarray(edge_index[0], np.int64)
    dst = np.asarray(edge_index[1], np.int64)
    attr = np.asarray(edge_attr, np.float32)

    p1 = params[0]
    We1 = np.asarray(p1["We"], np.float32)
    be1 = np.asarray(p1["be"], np.float32)
    sum1_all = x[src] + (attr @ We1 + be1)

    core_of = dst // npc

    per_core = []
    cpw1 = 1
    cpw_b = np.zeros(nbkt, np.int64)
    for c in range(n_cores):
        idx = np.nonzero(core_of == c)[0]
        dl = (dst[idx] - c * npc).astype(np.int64)
        bk = src[idx] // BKT
        order = np.lexsort((dl, bk))
        idx, dl, bk = idx[order], dl[order], bk[order]
        win = dl // P
        key = bk * nwin + win
        counts_bw = np.bincount(key, minlength=nbkt * nwin).reshape(nbkt, nwin)
        cpw_b = np.maximum(cpw_b, _ceil_div(counts_bw, P).max(axis=1))
        counts_w = counts_bw.sum(axis=0)
        cpw1 = max(cpw1, int(_ceil_div(counts_w.max(), P)))
        per_core.append((idx, dl, bk, win, counts_bw))

    cpw_b = np.maximum(cpw_b, 1).astype(np.int64)
    nchunk1 = nwin * cpw1
    nchunk2 = int(nwin * cpw_b.sum())
    cbase = np.zeros(nbkt + 1, np.int64)
    cbase[1:] = np.cumsum(nwin * cpw_b)

    in_maps = []
    for c in range(n_cores):
        idx, dl, bk, win, counts_bw = per_core[c]
        ne = len(idx)

        # ---- layer-1 (window-major) slots ----
        o1 = np.argsort(dl, kind="stable")
        i1, d1 = idx[o1], dl[o1]
        w1 = d1 // P
        cw = counts_bw.sum(axis=0)
        starts = np.zeros(nwin, np.int64)
        starts[1:] = np.cumsum(cw)[:-1]
        within1 = np.arange(ne) - starts[w1]
        slot1 = w1 * (cpw1 * P) + within1
        dstl1_pad = np.full(nchunk1 * P, -1.0, np.float32)
        sum1_pad = np.zeros((nchunk1 * P, in_dim), np.float32)
        dstl1_pad[slot1] = (d1 - w1 * P).astype(np.float32)
        sum1_pad[slot1] = sum1_all[i1]

        # ---- layers-2/3 (bucket-major) slots ----
        flat = counts_bw.flatten()
        sb_flat = np.zeros(nbkt * nwin, np.int64)
        sb_flat[1:] = np.cumsum(flat)[:-1]
        sb = sb_flat.reshape(nbkt, nwin)
        within2 = np.arange(ne) - sb[bk, win]
        slot2 = (cbase[bk] + win * cpw_b[bk]) * P + within2

        src16_pad = np.zeros(nchunk2 * P, np.int16)
        dstl2_pad = np.full(nchunk2 * P, -1.0, np.float32)
        attr_pad = np.zeros((nchunk2 * P, 2), np.float32)
        src16_pad[slot2] = (src[idx] - bk * BKT).astype(np.int16)
        dstl2_pad[slot2] = (dl - win * P).astype(np.float32)
        attr_pad[slot2] = attr[idx]

        ids = src16_pad.reshape(-1, 16)  # [S/16, 16]
        idx_dev = np.tile(ids.T, (8, 1))  # [128, S/16]

        im = {
            "idx16": np.ascontiguousarray(idx_dev),
            "dstl1": np.ascontiguousarray(
                dstl1_pad.reshape(nchunk1, P).T.astype(ml_dtypes.bfloat16)
            ),
            "dstl2": np.ascontiguousarray(
                dstl2_pad.reshape(nchunk2, P).T.astype(ml_dtypes.bfloat16)
            ),
            "attr3": np.ascontiguousarray(
                np.vstack(
                    [attr_pad[:, 0], attr_pad[:, 1], np.ones(nchunk2 * P, np.float32)]
                ).astype(ml_dtypes.bfloat16)
            ),
            "sum1": np.ascontiguousarray(
                sum1_pad.reshape(nchunk1, P, in_dim)
                .transpose(1, 0, 2)
                .reshape(P, nchunk1 * in_dim)
            ),
            "xT1": np.ascontiguousarray(x[c * npc : (c + 1) * npc].T),
        }
        for li in (1, 2):
            pl = params[li]
            im[f"We3_{li + 1}"] = np.ascontiguousarray(
                np.vstack(
                    [
                        np.asarray(pl["We"], np.float32),
                        np.asarray(pl["be"], np.float32)[None, :],
                    ]
                ).astype(ml_dtypes.bfloat16)
            )
        for li in range(3):
            pl = params[li]
            im[f"W{li + 1}"] = np.ascontiguousarray(np.asarray(pl["W"], np.float32))
            b = np.asarray(pl["b"], np.float32).reshape(hid, 1)
            im[f"b{li + 1}"] = np.ascontiguousarray(b)
            im[f"nb{li + 1}"] = np.ascontiguousarray(-b)
        in_maps.append(im)

    return in_maps, cpw1, cpw_b


def _build(cfg, cpw1, cpw_b):
    n_nodes = cfg["n_nodes"]
    n_cores = cfg["n_cores"]
    in_dim = cfg["in_dim"]
    hid = cfg["hid"]
    npc = n_nodes // n_cores
    nwin = _ceil_div(npc, P)
    nbkt = _ceil_div(n_nodes, BKT)
    nchunk1 = nwin * cpw1
    nchunk2 = int(nwin * cpw_b.sum())
    cbase = np.zeros(nbkt + 1, np.int64)
    cbase[1:] = np.cumsum(nwin * cpw_b)
    in_dims = [in_dim, hid, hid]

    nc = bacc.Bacc(num_devices=n_cores, num_swdge_queues=4)

    idx16_d = nc.dram_tensor("idx16", [P, nchunk2 * 8], I16, kind="ExternalInput")
    dstl1_d = nc.dram_tensor("dstl1", [P, nchunk1], BF16, kind="ExternalInput")
    dstl2_d = nc.dram_tensor("dstl2", [P, nchunk2], BF16, kind="ExternalInput")
    attr3_d = nc.dram_tensor("attr3", [3, nchunk2 * P], BF16, kind="ExternalInput")
    sum1_d = nc.dram_tensor("sum1", [P, nchunk1 * in_dim], F32, kind="ExternalInput")
    xT1_d = nc.dram_tensor("xT1", [in_dim, npc], F32, kind="ExternalInput")
    We3_d = {
        li: nc.dram_tensor(f"We3_{li + 1}", [3, hid], BF16, kind="ExternalInput")
        for li in (1, 2)
    }
    W_d, b_d, nb_d = {}, {}, {}
    for li in range(3):
        W_d[li] = nc.dram_tensor(
            f"W{li + 1}", [in_dims[li], hid], F32, kind="ExternalInput"
        )
        b_d[li] = nc.dram_tensor(f"b{li + 1}", [hid, 1], F32, kind="ExternalInput")
        nb_d[li] = nc.dram_tensor(f"nb{li + 1}", [hid, 1], F32, kind="ExternalInput")

    h_own = {li: nc.dram_tensor(f"h{li + 1}_own", [npc, hid], F32) for li in (0, 1)}
    h_full = {
        li: nc.dram_tensor(f"h{li + 1}_full", [n_nodes, hid], F32, addr_space="Shared")
        for li in (0, 1)
    }
    out_d = nc.dram_tensor("out", [npc, hid], F32, kind="ExternalOutput")

    rg = [list(range(n_cores))]

    with tile.TileContext(nc) as tc:
        with (
            tc.tile_pool(name="const", bufs=1) as cpool,
            tc.tile_pool(name="gath", bufs=2) as gpool,
            tc.tile_pool(name="edata", bufs=3) as epool,
            tc.tile_pool(name="work", bufs=3) as wpool,
            tc.tile_pool(name="oneh", bufs=4) as opool,
            tc.tile_pool(name="zt", bufs=2) as zpool,
            tc.tile_pool(name="uv", bufs=2) as uvpool,
            tc.tile_pool(name="trs", bufs=2) as trpool,
            tc.tile_pool(name="eps", bufs=2, space="PSUM") as eps_pool,
            tc.tile_pool(name="aggp", bufs=2, space="PSUM") as agg_pool,
            tc.tile_pool(name="pbp", bufs=2, space="PSUM") as pb_pool,
        ):
            # ---- resident constants ----
            dstl1 = cpool.tile([P, nchunk1], BF16)
            dstl2 = cpool.tile([P, nchunk2], BF16)
            nc.sync.dma_start(dstl1[:], dstl1_d[:])
            nc.sync.dma_start(dstl2[:], dstl2_d[:])

            iota_i = cpool.tile([P, P], I32)
            nc.gpsimd.iota(iota_i[:], pattern=[[1, P]], base=0, channel_multiplier=0)
            iota_b = cpool.tile([P, P], BF16)
            nc.vector.tensor_copy(iota_b[:], iota_i[:])
            ident = cpool.tile([P, P], F32)
            make_identity(nc, ident[:])
            # absorb cross-engine waits ahead of the steady-state loops
            absorb = cpool.tile([P, 1], BF16)
            nc.vector.tensor_tensor(
                out=absorb[:],
                in0=dstl1[:, 0:1],
                in1=iota_b[:, 0:1],
                op=mybir.AluOpType.add,
            )
            absorb2 = cpool.tile([P, 1], BF16)
            nc.vector.tensor_tensor(
                out=absorb2[:],
                in0=dstl2[:, 0:1],
                in1=iota_b[:, 0:1],
                op=mybir.AluOpType.add,
            )

            hT_own = cpool.tile([P, npc], F32)
            aggT_sb = cpool.tile([P, npc], F32)

            We3_t = {}
            for li in (1, 2):
                We3_t[li] = cpool.tile(
                    [3, hid], BF16, tag=f"we3_{li}", name=f"we3_{li}"
                )
                nc.sync.dma_start(We3_t[li][:], We3_d[li][:])
            W_t, b_t, nb_t = {}, {}, {}
            for li in range(3):
                W_t[li] = cpool.tile(
                    [in_dims[li], hid], F32, tag=f"w_{li}", name=f"w_{li}"
                )
                nc.sync.dma_start(W_t[li][:], W_d[li][:])
                b_t[li] = cpool.tile([hid, 1], F32, tag=f"b_{li}", name=f"b_{li}")
                nc.sync.dma_start(b_t[li][:], b_d[li][:])
                nb_t[li] = cpool.tile([hid, 1], F32, tag=f"nb_{li}", name=f"nb_{li}")
                nc.sync.dma_start(nb_t[li][:], nb_d[li][:])

            wins = []
            for w in range(nwin):
                w0 = w * P
                wins.append((w, w0, min(P, npc - w0)))
            groups = [wins[i : i + GRP] for i in range(0, nwin, GRP)]

            def node_phase(li, g, zT_s, dst_dram):
                g0 = g[0][1]
                gn = sum(wn for _, _, wn in g)
                K = in_dims[li]
                hT_ps = pb_pool.tile([P, GRP * P], F32, tag="pb", name="hT_ps")
                nc.tensor.matmul(
                    hT_ps[:, :gn],
                    lhsT=W_t[li][:, :],
                    rhs=zT_s[:K, :gn],
                    start=True,
                    stop=True,
                )
                v = uvpool.tile([P, GRP * P], F32, tag="v", name="v")
                nc.scalar.activation(
                    out=hT_own[:, g0 : g0 + gn],
                    in_=hT_ps[:, :gn],
                    func=mybir.ActivationFunctionType.Relu,
                    bias=b_t[li][:],
                    scale=1.0,
                )
                nc.scalar.activation(
                    out=v[:, :gn],
                    in_=hT_ps[:, :gn],
                    func=mybir.ActivationFunctionType.Relu,
                    bias=nb_t[li][:],
                    scale=-1.0,
                )
                nc.vector.tensor_scalar(
                    out=v[:, :gn],
                    in0=v[:, :gn],
                    scalar1=-0.01,
                    scalar2=None,
                    op0=mybir.AluOpType.mult,
                )
                nc.vector.tensor_tensor(
                    out=hT_own[:, g0 : g0 + gn],
                    in0=hT_own[:, g0 : g0 + gn],
                    in1=v[:, :gn],
                    op=mybir.AluOpType.add,
                )
                tr_ps = pb_pool.tile([P, GRP * P], F32, tag="pb", name="tr_ps")
                tr_sb = trpool.tile([P, GRP * P], F32)
                for i, (w, w0, wn) in enumerate(g):
                    nc.tensor.transpose(
                        out=tr_ps[:wn, i * P : i * P + P],
                        in_=hT_own[:, w0 : w0 + wn],
                        identity=ident[:],
                    )
                nc.vector.tensor_copy(tr_sb[:, : len(g) * P], tr_ps[:, : len(g) * P])
                for i, (w, w0, wn) in enumerate(g):
                    nc.sync.dma_start(
                        out=dst_dram[w0 : w0 + wn, :],
                        in_=tr_sb[:wn, i * P : i * P + P],
                    )

            # ================= layer 1 =================
            for g in groups:
                gc0 = g[0][0] * cpw1
                gcn = len(g) * cpw1
                sum1_g = epool.tile(
                    [P, GRP * cpw1 * in_dim], F32, tag="sum1", name="sum1_g"
                )
                nc.sync.dma_start(
                    sum1_g[:, : gcn * in_dim],
                    sum1_d[:, gc0 * in_dim : (gc0 + gcn) * in_dim],
                )
                msg1_g = epool.tile(
                    [P, GRP * cpw1 * in_dim], BF16, tag="msg1", name="msg1_g"
                )
                nc.scalar.activation(
                    out=msg1_g[:, : gcn * in_dim],
                    in_=sum1_g[:, : gcn * in_dim],
                    func=mybir.ActivationFunctionType.Relu,
                )
                zT_s = zpool.tile([P, GRP * P], F32)
                g0 = g[0][1]
                gn = sum(wn for _, _, wn in g)
                xT1_g = epool.tile([in_dim, GRP * P], F32, tag="xt1", name="xT1_g")
                nc.sync.dma_start(xT1_g[:, :gn], xT1_d[:, g0 : g0 + gn])
                for gi, (w, w0, wn) in enumerate(g):
                    agg_ps = agg_pool.tile([in_dim, P], F32, tag="agg1", name="agg1")
                    for k in range(cpw1):
                        c = w * cpw1 + k
                        lc = gi * cpw1 + k
                        oh = opool.tile([P, P], BF16)
                        nc.vector.tensor_tensor(
                            out=oh[:],
                            in0=iota_b[:],
                            in1=dstl1[:, c : c + 1].to_broadcast([P, P]),
                            op=mybir.AluOpType.is_equal,
                        )
                        nc.tensor.matmul(
                            agg_ps[:],
                            lhsT=msg1_g[:, lc * in_dim : (lc + 1) * in_dim],
                            rhs=oh[:],
                            start=(k == 0),
                            stop=(k == cpw1 - 1),
                        )
                    nc.vector.tensor_tensor(
                        out=zT_s[:in_dim, gi * P : gi * P + wn],
                        in0=agg_ps[:, :wn],
                        in1=xT1_g[:, gi * P : gi * P + wn],
                        op=mybir.AluOpType.add,
                    )
                node_phase(0, g, zT_s, h_own[0])

            nc.gpsimd.collective_compute(
                "AllGather",
                mybir.AluOpType.bypass,
                replica_groups=rg,
                ins=[h_own[0][:]],
                outs=[h_full[0][:]],
            )

            # ================= layers 2, 3 =================
            for li in (1, 2):
                h_src = h_full[li - 1]
                dst_dram = h_own[1] if li == 1 else out_d

                agg_open = {}  # w -> open psum accumulation tile
                gq = 0
                for b in range(nbkt):
                    cb = int(cpw_b[b])
                    base = int(cbase[b])
                    nck = nwin * cb
                    tbl = h_src[b * BKT : min((b + 1) * BKT, n_nodes), :]
                    for gi0 in range(0, nck, GB):
                        gcn = min(GB, nck - gi0)
                        c0 = base + gi0
                        idx_t = wpool.tile([P, GB * 8], I16, tag="idx", name="idx_t")
                        nc.sync.dma_start(
                            idx_t[:, : gcn * 8],
                            idx16_d[:, c0 * 8 : (c0 + gcn) * 8],
                        )
                        gth = gpool.tile([P, GB, P], F32, name="gth")
                        nc.gpsimd.dma_gather(
                            out_ap=gth[:, :gcn, :],
                            in_ap=tbl,
                            idxs_ap=idx_t[:, : gcn * 8],
                            num_idxs=gcn * P,
                            num_idxs_reg=gcn * P,
                            elem_size=P,
                            queue_num=gq % 4,
                        )
                        gq += 1
                        for e0 in range(0, gcn, EPB):
                            en = min(EPB, gcn - e0)
                            attr3_g = epool.tile(
                                [3, EPB * P], BF16, tag="attr3", name="attr3_g"
                            )
                            nc.sync.dma_start(
                                attr3_g[:, : en * P],
                                attr3_d[:, (c0 + e0) * P : (c0 + e0 + en) * P],
                            )
                            e_ps = eps_pool.tile(
                                [P, EPB * P], F32, tag="eps", name="e_ps"
                            )
                            for j in range(en):
                                nc.tensor.matmul(
                                    e_ps[:, j * P : (j + 1) * P],
                                    lhsT=attr3_g[:, j * P : (j + 1) * P],
                                    rhs=We3_t[li][:],
                                    start=True,
                                    stop=True,
                                )
                            s_sb = wpool.tile([P, EPB * P], F32, tag="s", name="s_sb")
                            nc.vector.tensor_tensor(
                                out=s_sb[:, : en * P],
                                in0=gth[:, e0 : e0 + en, :].rearrange(
                                    "p c d -> p (c d)"
                                ),
                                in1=e_ps[:, : en * P],
                                op=mybir.AluOpType.add,
                            )
                            msg = wpool.tile([P, EPB * P], BF16, tag="m", name="msg")
                            nc.scalar.activation(
                                out=msg[:, : en * P],
                                in_=s_sb[:, : en * P],
                                func=mybir.ActivationFunctionType.Relu,
                            )
                            for j in range(en):
                                k = gi0 + e0 + j
                                c = c0 + e0 + j
                                w, kk = divmod(k, cb)
                                if kk == 0:
                                    agg_open[w] = agg_pool.tile(
                                        [P, P], F32, tag="agg", name="agg_ps"
                                    )
                                oh = opool.tile([P, P], BF16)
                                nc.vector.tensor_tensor(
                                    out=oh[:],
                                    in0=iota_b[:],
                                    in1=dstl2[:, c : c + 1].to_broadcast([P, P]),
                                    op=mybir.AluOpType.is_equal,
                                )
                                nc.tensor.matmul(
                                    agg_open[w][:],
                                    lhsT=msg[:, j * P : (j + 1) * P],
                                    rhs=oh[:],
                                    start=(kk == 0),
                                    stop=(kk == cb - 1),
                                )
                                if kk == cb - 1:
                                    w0 = w * P
                                    wn = min(P, npc - w0)
                                    if b == 0:
                                        nc.vector.tensor_copy(
                                            aggT_sb[:, w0 : w0 + wn],
                                            agg_open[w][:, :wn],
                                        )
                                    else:
                                        nc.vector.tensor_tensor(
                                            out=aggT_sb[:, w0 : w0 + wn],
                                            in0=aggT_sb[:, w0 : w0 + wn],
                                            in1=agg_open[w][:, :wn],
                                            op=mybir.AluOpType.add,
                                        )
                                    del agg_open[w]

                # node phase sweep
                for g in groups:
                    g0 = g[0][1]
                    gn = sum(wn for _, _, wn in g)
                    zT_s = zpool.tile([P, GRP * P], F32)
                    nc.vector.tensor_tensor(
                        out=zT_s[:, :gn],
                        in0=aggT_sb[:, g0 : g0 + gn],
                        in1=hT_own[:, g0 : g0 + gn],
                        op=mybir.AluOpType.add,
                    )
                    node_phase(li, g, zT_s, dst_dram)

                if li == 1:
                    nc.gpsimd.collective_compute(
                        "AllGather",
                        mybir.AluOpType.bypass,
                        replica_groups=rg,
                        ins=[h_own[1][:]],
                        outs=[h_full[1][:]],
                    )

    nc.compile()
    return nc


def kernel(x, edge_index, edge_attr, params):
    from concourse.bass_utils import run_bass_kernel_spmd

    cfg = CFG
    in_maps, cpw1, cpw_b = _host_prep(x, edge_index, edge_attr, params, cfg)
    nc = _build(cfg, cpw1, cpw_b)
    res = run_bass_kernel_spmd(nc, in_maps, list(range(cfg["n_cores"])))
    return np.concatenate([r["out"] for r in res.results], axis=0)


# revision 11
# speedup vs baseline: 1.0794x; 1.0794x over previous
"""GINEConv x3 GNN backbone on 8 Trainium2 NeuronCores.

Strategy (graph-parallel by destination node, per the sharding hint):
- Nodes split contiguously across 8 cores (12500 each). Each core owns all
  edges whose destination is in its range, so segment-sum is core-local.
- Edge phase per chunk of 128 edges (layers 2/3):
    gather x[src]   : bulk dma_gather (q7 ucode; int16 idx) - node ids are
                      bucketed into 32768-row table slices to fit int16
    e = attr @ We+be: PE matmul, lhsT=attr3 [3,128], rhs=We3 [3,128]
    s = x_src + e   : DVE tensor_tensor add (PSUM + SBUF)
    msg = relu(s)   : ACT (bf16 out)
    onehot[dst]     : DVE tensor_tensor is_equal vs iota row (bf16)
    aggT += msg.T @ onehot : PE matmul accumulating in PSUM, feature-major,
                      flushed to an SBUF accumulator per (bucket, window)
- Node phase per 4-window group: zT = aggT + hT_own; hT = W.T @ zT (PE);
  leaky-relu = relu(z+b) - 0.01*relu(-z-b) (ACT+DVE); PE transpose to
  node-major; DMA to DRAM.
- AllGather of h after layers 1 and 2 so every core can gather any node row.
- Layer 1 (in_dim=5): host precomputes the 5-dim pre-activation messages
  x[src] + lin_edge(attr); device does relu + scatter + node phase.
- Edges are padded so every (bucket, window) run is a whole number of
  128-edge chunks, with the same chunk count on every core (single SPMD
  program). Pad edges gather row 0 and carry dst_local=-1 so their one-hot
  column is all zeros.
"""

import ml_dtypes
import numpy as np

import concourse.bacc as bacc
import concourse.bass as bass
import concourse.mybir as mybir
import concourse.tile as tile
from concourse.masks import make_identity

P = 128
BKT = 32768  # int16 bucket size for dma_gather indices
GB = 8  # chunks per dma_gather call (1024 idx = HW-validated max)
F32 = mybir.dt.float32
BF16 = mybir.dt.bfloat16
I16 = mybir.dt.int16
I32 = mybir.dt.int32

CFG = dict(n_nodes=100000, n_edges=1600000, in_dim=5, hid=128, n_cores=8)

GRP = 4  # windows per node-phase group
EPB = 4  # chunks per e-matmul/add/relu batch


def _ceil_div(a, b):
    return (a + b - 1) // b


def _host_prep(x, edge_index, edge_attr, params, cfg):
    """Returns (in_maps, cpw1, cpw_b)."""
    n_nodes = cfg["n_nodes"]
    n_cores = cfg["n_cores"]
    in_dim = cfg["in_dim"]
    hid = cfg["hid"]
    npc = n_nodes // n_cores
    nwin = _ceil_div(npc, P)
    nbkt = _ceil_div(n_nodes, BKT)

    x = np.asarray(x, np.float32)
    src = np.asarray(edge_index[0], np.int64)
    dst = np.asarray(edge_index[1], np.int64)
    attr = np.asarray(edge_attr, np.float32)

    p1 = params[0]
    We1 = np.asarray(p1["We"], np.float32)
    be1 = np.asarray(p1["be"], np.float32)
    sum1_all = x[src] + (attr @ We1 + be1)

    core_of = dst // npc

    per_core = []
    cpw1 = 1
    cpw_b = np.zeros(nbkt, np.int64)
    for c in range(n_cores):
        idx = np.nonzero(core_of == c)[0]
        dl = (dst[idx] - c * npc).astype(np.int64)
        bk = src[idx] // BKT
        order = np.lexsort((dl, bk))
        idx, dl, bk = idx[order], dl[order], bk[order]
        win = dl // P
        key = bk * nwin + win
        counts_bw = np.bincount(key, minlength=nbkt * nwin).reshape(nbkt, nwin)
        cpw_b = np.maximum(cpw_b, _ceil_div(counts_bw, P).max(axis=1))
        counts_w = counts_bw.sum(axis=0)
        cpw1 = max(cpw1, int(_ceil_div(counts_w.max(), P)))
        per_core.append((idx, dl, bk, win, counts_bw))

    cpw_b = np.maximum(cpw_b, 1).astype(np.int64)
    nchunk1 = nwin * cpw1
    nchunk2 = int(nwin * cpw_b.sum())
    cbase = np.zeros(nbkt + 1, np.int64)
    cbase[1:] = np.cumsum(nwin * cpw_b)

    in_maps = []
    for c in range(n_cores):
        idx, dl, bk, win, counts_bw = per_core[c]
        ne = len(idx)

        # ---- layer-1 (window-major) slots ----
        o1 = np.argsort(dl, kind="stable")
        i1, d1 = idx[o1], dl[o1]
        w1 = d1 // P
        cw = counts_bw.sum(axis=0)
        starts = np.zeros(nwin, np.int64)
        starts[1:] = np.cumsum(cw)[:-1]
        within1 = np.arange(ne) - starts[w1]
        slot1 = w1 * (cpw1 * P) + within1
        dstl1_pad = np.full(nchunk1 * P, -1.0, np.float32)
        sum1_pad = np.zeros((nchunk1 * P, in_dim), np.float32)
        dstl1_pad[slot1] = (d1 - w1 * P).astype(np.float32)
        sum1_pad[slot1] = sum1_all[i1]

        # ---- layers-2/3 (bucket-major) slots ----
        flat = counts_bw.flatten()
        sb_flat = np.zeros(nbkt * nwin, np.int64)
        sb_flat[1:] = np.cumsum(flat)[:-1]
        sb = sb_flat.reshape(nbkt, nwin)
        within2 = np.arange(ne) - sb[bk, win]
        slot2 = (cbase[bk] + win * cpw_b[bk]) * P + within2

        src16_pad = np.zeros(nchunk2 * P, np.int16)
        dstl2_pad = np.full(nchunk2 * P, -1.0, np.float32)
        attr_pad = np.zeros((nchunk2 * P, 2), np.float32)
        src16_pad[slot2] = (src[idx] - bk * BKT).astype(np.int16)
        dstl2_pad[slot2] = (dl - win * P).astype(np.float32)
        attr_pad[slot2] = attr[idx]

        ids = src16_pad.reshape(-1, 16)  # [S/16, 16]
        idx_dev = np.tile(ids.T, (8, 1))  # [128, S/16]

        im = {
            "idx16": np.ascontiguousarray(idx_dev),
            "dstl1": np.ascontiguousarray(
                dstl1_pad.reshape(nchunk1, P).T.astype(ml_dtypes.bfloat16)
            ),
            "dstl2": np.ascontiguousarray(
                dstl2_pad.reshape(nchunk2, P).T.astype(ml_dtypes.bfloat16)
            ),
            "attr3": np.ascontiguousarray(
                np.vstack(
                    [attr_pad[:, 0], attr_pad[:, 1], np.ones(nchunk2 * P, np.float32)]
                ).astype(ml_dtypes.bfloat16)
            ),
            "sum1": np.ascontiguousarray(
                sum1_pad.reshape(nchunk1, P, in_dim)
                .transpose(1, 0, 2)
                .reshape(P, nchunk1 * in_dim)
            ),
            "xT1": np.ascontiguousarray(x[c * npc : (c + 1) * npc].T),
        }
        for li in (1, 2):
            pl = params[li]
            im[f"We3_{li + 1}"] = np.ascontiguousarray(
                np.vstack(
                    [
                        np.asarray(pl["We"], np.float32),
                        np.asarray(pl["be"], np.float32)[None, :],
                    ]
                ).astype(ml_dtypes.bfloat16)
            )
        for li in range(3):
            pl = params[li]
            im[f"W{li + 1}"] = np.ascontiguousarray(np.asarray(pl["W"], np.float32))
            b = np.asarray(pl["b"], np.float32).reshape(hid, 1)
            im[f"b{li + 1}"] = np.ascontiguousarray(b)
            im[f"nb{li + 1}"] = np.ascontiguousarray(-b)
        in_maps.append(im)

    return in_maps, cpw1, cpw_b


def _build(cfg, cpw1, cpw_b):
    n_nodes = cfg["n_nodes"]
    n_cores = cfg["n_cores"]
    in_dim = cfg["in_dim"]
    hid = cfg["hid"]
    npc = n_nodes // n_cores
    nwin = _ceil_div(npc, P)
    nbkt = _ceil_div(n_nodes, BKT)
    nchunk1 = nwin * cpw1
    nchunk2 = int(nwin * cpw_b.sum())
    cbase = np.zeros(nbkt + 1, np.int64)
    cbase[1:] = np.cumsum(nwin * cpw_b)
    in_dims = [in_dim, hid, hid]

    nc = bacc.Bacc(num_devices=n_cores, num_swdge_queues=4)

    idx16_d = nc.dram_tensor("idx16", [P, nchunk2 * 8], I16, kind="ExternalInput")
    dstl1_d = nc.dram_tensor("dstl1", [P, nchunk1], BF16, kind="ExternalInput")
    dstl2_d = nc.dram_tensor("dstl2", [P, nchunk2], BF16, kind="ExternalInput")
    attr3_d = nc.dram_tensor("attr3", [3, nchunk2 * P], BF16, kind="ExternalInput")
    sum1_d = nc.dram_tensor("sum1", [P, nchunk1 * in_dim], F32, kind="ExternalInput")
    xT1_d = nc.dram_tensor("xT1", [in_dim, npc], F32, kind="ExternalInput")
    We3_d = {
        li: nc.dram_tensor(f"We3_{li + 1}", [3, hid], BF16, kind="ExternalInput")
        for li in (1, 2)
    }
    W_d, b_d, nb_d = {}, {}, {}
    for li in range(3):
        W_d[li] = nc.dram_tensor(
            f"W{li + 1}", [in_dims[li], hid], F32, kind="ExternalInput"
        )
        b_d[li] = nc.dram_tensor(f"b{li + 1}", [hid, 1], F32, kind="ExternalInput")
        nb_d[li] = nc.dram_tensor(f"nb{li + 1}", [hid, 1], F32, kind="ExternalInput")

    h_own = {li: nc.dram_tensor(f"h{li + 1}_own", [npc, hid], F32) for li in (0, 1)}
    h_full = {
        li: nc.dram_tensor(f"h{li + 1}_full", [n_nodes, hid], F32, addr_space="Shared")
        for li in (0, 1)
    }
    out_d = nc.dram_tensor("out", [npc, hid], F32, kind="ExternalOutput")

    rg = [list(range(n_cores))]

    with tile.TileContext(nc) as tc:
        with (
            tc.tile_pool(name="const", bufs=1) as cpool,
            tc.tile_pool(name="gath", bufs=4) as gpool,
            tc.tile_pool(name="idxp", bufs=6) as ixpool,
            tc.tile_pool(name="edata", bufs=3) as epool,
            tc.tile_pool(name="work", bufs=3) as wpool,
            tc.tile_pool(name="oneh", bufs=4) as opool,
            tc.tile_pool(name="zt", bufs=2) as zpool,
            tc.tile_pool(name="uv", bufs=2) as uvpool,
            tc.tile_pool(name="trs", bufs=2) as trpool,
            tc.tile_pool(name="eps", bufs=2, space="PSUM") as eps_pool,
            tc.tile_pool(name="aggp", bufs=2, space="PSUM") as agg_pool,
            tc.tile_pool(name="pbp", bufs=2, space="PSUM") as pb_pool,
        ):
            # ---- resident constants ----
            dstl1 = cpool.tile([P, nchunk1], BF16)
            dstl2 = cpool.tile([P, nchunk2], BF16)
            nc.sync.dma_start(dstl1[:], dstl1_d[:])
            nc.sync.dma_start(dstl2[:], dstl2_d[:])

            iota_i = cpool.tile([P, P], I32)
            nc.gpsimd.iota(iota_i[:], pattern=[[1, P]], base=0, channel_multiplier=0)
            iota_b = cpool.tile([P, P], BF16)
            nc.vector.tensor_copy(iota_b[:], iota_i[:])
            ident = cpool.tile([P, P], F32)
            make_identity(nc, ident[:])
            # absorb cross-engine waits ahead of the steady-state loops
            absorb = cpool.tile([P, 1], BF16)
            nc.vector.tensor_tensor(
                out=absorb[:],
                in0=dstl1[:, 0:1],
                in1=iota_b[:, 0:1],
                op=mybir.AluOpType.add,
            )
            absorb2 = cpool.tile([P, 1], BF16)
            nc.vector.tensor_tensor(
                out=absorb2[:],
                in0=dstl2[:, 0:1],
                in1=iota_b[:, 0:1],
                op=mybir.AluOpType.add,
            )

            hT_own = cpool.tile([P, npc], F32)
            aggT_sb = cpool.tile([P, npc], F32)

            We3_t = {}
            for li in (1, 2):
                We3_t[li] = cpool.tile(
                    [3, hid], BF16, tag=f"we3_{li}", name=f"we3_{li}"
                )
                nc.sync.dma_start(We3_t[li][:], We3_d[li][:])
            W_t, b_t, nb_t = {}, {}, {}
            for li in range(3):
                W_t[li] = cpool.tile(
                    [in_dims[li], hid], F32, tag=f"w_{li}", name=f"w_{li}"
                )
                nc.sync.dma_start(W_t[li][:], W_d[li][:])
                b_t[li] = cpool.tile([hid, 1], F32, tag=f"b_{li}", name=f"b_{li}")
                nc.sync.dma_start(b_t[li][:], b_d[li][:])
                nb_t[li] = cpool.tile([hid, 1], F32, tag=f"nb_{li}", name=f"nb_{li}")
                nc.sync.dma_start(nb_t[li][:], nb_d[li][:])

            wins = []
            for w in range(nwin):
                w0 = w * P
                wins.append((w, w0, min(P, npc - w0)))
            groups = [wins[i : i + GRP] for i in range(0, nwin, GRP)]

            def node_phase(li, g, zT_s, dst_dram):
                g0 = g[0][1]
                gn = sum(wn for _, _, wn in g)
                K = in_dims[li]
                hT_ps = pb_pool.tile([P, GRP * P], F32, tag="pb", name="hT_ps")
                nc.tensor.matmul(
                    hT_ps[:, :gn],
                    lhsT=W_t[li][:, :],
                    rhs=zT_s[:K, :gn],
                    start=True,
                    stop=True,
                )
                v = uvpool.tile([P, GRP * P], F32, tag="v", name="v")
                nc.scalar.activation(
                    out=hT_own[:, g0 : g0 + gn],
                    in_=hT_ps[:, :gn],
                    func=mybir.ActivationFunctionType.Relu,
                    bias=b_t[li][:],
                    scale=1.0,
                )
                nc.scalar.activation(
                    out=v[:, :gn],
                    in_=hT_ps[:, :gn],
                    func=mybir.ActivationFunctionType.Relu,
                    bias=nb_t[li][:],
                    scale=-1.0,
                )
                nc.vector.tensor_scalar(
                    out=v[:, :gn],
                    in0=v[:, :gn],
                    scalar1=-0.01,
                    scalar2=None,
                    op0=mybir.AluOpType.mult,
                )
                nc.vector.tensor_tensor(
                    out=hT_own[:, g0 : g0 + gn],
                    in0=hT_own[:, g0 : g0 + gn],
                    in1=v[:, :gn],
                    op=mybir.AluOpType.add,
                )
                tr_ps = pb_pool.tile([P, GRP * P], F32, tag="pb", name="tr_ps")
                tr_sb = trpool.tile([P, GRP * P], F32)
                for i, (w, w0, wn) in enumerate(g):
                    nc.tensor.transpose(
                        out=tr_ps[:wn, i * P : i * P + P],
                        in_=hT_own[:, w0 : w0 + wn],
                        identity=ident[:],
                    )
                nc.vector.tensor_copy(tr_sb[:, : len(g) * P], tr_ps[:, : len(g) * P])
                for i, (w, w0, wn) in enumerate(g):
                    nc.sync.dma_start(
                        out=dst_dram[w0 : w0 + wn, :],
                        in_=tr_sb[:wn, i * P : i * P + P],
                    )

            # ================= layer 1 =================
            for g in groups:
                gc0 = g[0][0] * cpw1
                gcn = len(g) * cpw1
                sum1_g = epool.tile(
                    [P, GRP * cpw1 * in_dim], F32, tag="sum1", name="sum1_g"
                )
                nc.sync.dma_start(
                    sum1_g[:, : gcn * in_dim],
                    sum1_d[:, gc0 * in_dim : (gc0 + gcn) * in_dim],
                )
                msg1_g = epool.tile(
                    [P, GRP * cpw1 * in_dim], BF16, tag="msg1", name="msg1_g"
                )
                nc.scalar.activation(
                    out=msg1_g[:, : gcn * in_dim],
                    in_=sum1_g[:, : gcn * in_dim],
                    func=mybir.ActivationFunctionType.Relu,
                )
                zT_s = zpool.tile([P, GRP * P], F32)
                g0 = g[0][1]
                gn = sum(wn for _, _, wn in g)
                xT1_g = epool.tile([in_dim, GRP * P], F32, tag="xt1", name="xT1_g")
                nc.sync.dma_start(xT1_g[:, :gn], xT1_d[:, g0 : g0 + gn])
                for gi, (w, w0, wn) in enumerate(g):
                    agg_ps = agg_pool.tile([in_dim, P], F32, tag="agg1", name="agg1")
                    for k0 in range(0, cpw1, EPB):
                        kn = min(EPB, cpw1 - k0)
                        c = w * cpw1 + k0
                        ohb = opool.tile([P, EPB, P], BF16)
                        nc.vector.tensor_tensor(
                            out=ohb[:, :kn, :],
                            in0=iota_b[:].rearrange("p (o d) -> p o d", o=1).to_broadcast(
                                [P, kn, P]
                            ),
                            in1=dstl1[:, c : c + kn].to_broadcast([P, kn, P]),
                            op=mybir.AluOpType.is_equal,
                        )
                        for j in range(kn):
                            k = k0 + j
                            lc = gi * cpw1 + k
                            nc.tensor.matmul(
                                agg_ps[:],
                                lhsT=msg1_g[:, lc * in_dim : (lc + 1) * in_dim],
                                rhs=ohb[:, j, :],
                                start=(k == 0),
                                stop=(k == cpw1 - 1),
                            )
                    nc.vector.tensor_tensor(
                        out=zT_s[:in_dim, gi * P : gi * P + wn],
                        in0=agg_ps[:, :wn],
                        in1=xT1_g[:, gi * P : gi * P + wn],
                        op=mybir.AluOpType.add,
                    )
                node_phase(0, g, zT_s, h_own[0])

            nc.gpsimd.collective_compute(
                "AllGather",
                mybir.AluOpType.bypass,
                replica_groups=rg,
                ins=[h_own[0][:]],
                outs=[h_full[0][:]],
            )

            # ================= layers 2, 3 =================
            for li in (1, 2):
                h_src = h_full[li - 1]
                dst_dram = h_own[1] if li == 1 else out_d

                agg_open = {}  # w -> open psum accumulation tile
                gq = 0
                for b in range(nbkt):
                    cb = int(cpw_b[b])
                    base = int(cbase[b])
                    nck = nwin * cb
                    tbl = h_src[b * BKT : min((b + 1) * BKT, n_nodes), :]
                    for gi0 in range(0, nck, GB):
                        gcn = min(GB, nck - gi0)
                        c0 = base + gi0
                        idx_t = ixpool.tile([P, GB * 8], I16, tag="idx", name="idx_t")
                        nc.sync.dma_start(
                            idx_t[:, : gcn * 8],
                            idx16_d[:, c0 * 8 : (c0 + gcn) * 8],
                        )
                        gth = gpool.tile([P, GB, P], F32, name="gth")
                        nc.gpsimd.dma_gather(
                            out_ap=gth[:, :gcn, :],
                            in_ap=tbl,
                            idxs_ap=idx_t[:, : gcn * 8],
                            num_idxs=gcn * P,
                            num_idxs_reg=gcn * P,
                            elem_size=P,
                            queue_num=gq % 4,
                        )
                        gq += 1
                        for e0 in range(0, gcn, EPB):
                            en = min(EPB, gcn - e0)
                            attr3_g = epool.tile(
                                [3, EPB * P], BF16, tag="attr3", name="attr3_g"
                            )
                            nc.sync.dma_start(
                                attr3_g[:, : en * P],
                                attr3_d[:, (c0 + e0) * P : (c0 + e0 + en) * P],
                            )
                            e_ps = eps_pool.tile(
                                [P, EPB * P], F32, tag="eps", name="e_ps"
                            )
                            for j in range(en):
                                nc.tensor.matmul(
                                    e_ps[:, j * P : (j + 1) * P],
                                    lhsT=attr3_g[:, j * P : (j + 1) * P],
                                    rhs=We3_t[li][:],
                                    start=True,
                                    stop=True,
                                )
                            s_sb = wpool.tile([P, EPB * P], F32, tag="s", name="s_sb")
                            nc.vector.tensor_tensor(
                                out=s_sb[:, : en * P],
                                in0=gth[:, e0 : e0 + en, :].rearrange(
                                    "p c d -> p (c d)"
                                ),
                                in1=e_ps[:, : en * P],
                                op=mybir.AluOpType.add,
                            )
                            msg = wpool.tile([P, EPB * P], BF16, tag="m", name="msg")
                            nc.scalar.activation(
                                out=msg[:, : en * P],
                                in_=s_sb[:, : en * P],
                                func=mybir.ActivationFunctionType.Relu,
                            )
                            ohb = opool.tile([P, EPB, P], BF16)
                            c = c0 + e0
                            nc.vector.tensor_tensor(
                                out=ohb[:, :en, :],
                                in0=iota_b[:].rearrange("p (o d) -> p o d", o=1).to_broadcast(
                                    [P, en, P]
                                ),
                                in1=dstl2[:, c : c + en].to_broadcast([P, en, P]),
                                op=mybir.AluOpType.is_equal,
                            )
                            for j in range(en):
                                k = gi0 + e0 + j
                                w, kk = divmod(k, cb)
                                if kk == 0:
                                    agg_open[w] = agg_pool.tile(
                                        [P, P], F32, tag="agg", name="agg_ps"
                                    )
                                nc.tensor.matmul(
                                    agg_open[w][:],
                                    lhsT=msg[:, j * P : (j + 1) * P],
                                    rhs=ohb[:, j, :],
                                    start=(kk == 0),
                                    stop=(kk == cb - 1),
                                )
                                if kk == cb - 1:
                                    w0 = w * P
                                    wn = min(P, npc - w0)
                                    if b == 0:
                                        nc.vector.tensor_copy(
                                            aggT_sb[:, w0 : w0 + wn],
                                            agg_open[w][:, :wn],
                                        )
                                    else:
                                        nc.vector.tensor_tensor(
                                            out=aggT_sb[:, w0 : w0 + wn],
                                            in0=aggT_sb[:, w0 : w0 + wn],
                                            in1=agg_open[w][:, :wn],
                                            op=mybir.AluOpType.add,
                                        )
                                    del agg_open[w]

                # node phase sweep
                for g in groups:
                    g0 = g[0][1]
                    gn = sum(wn for _, _, wn in g)
                    zT_s = zpool.tile([P, GRP * P], F32)
                    nc.vector.tensor_tensor(
                        out=zT_s[:, :gn],
                        in0=aggT_sb[:, g0 : g0 + gn],
                        in1=hT_own[:, g0 : g0 + gn],
                        op=mybir.AluOpType.add,
                    )
                    node_phase(li, g, zT_s, dst_dram)

                if li == 1:
                    nc.gpsimd.collective_compute(
                        "AllGather",
                        mybir.AluOpType.bypass,
                        replica_groups=rg,
                        ins=[h_own[1][:]],
                        outs=[h_full[1][:]],
                    )

    nc.compile()
    return nc


def kernel(x, edge_index, edge_attr, params):
    from concourse.bass_utils import run_bass_kernel_spmd

    cfg = CFG
    in_maps, cpw1, cpw_b = _host_prep(x, edge_index, edge_attr, params, cfg)
    nc = _build(cfg, cpw1, cpw_b)
    res = run_bass_kernel_spmd(nc, in_maps, list(range(cfg["n_cores"])))
    return np.concatenate([r["out"] for r in res.results], axis=0)
